# revision 3
# baseline (speedup 1.0000x reference)
"""CodonAttention Trainium2 kernel (fp16 stream, per-chunk pipeline).

Math (per batch b, head h):
  q = x @ wq.T + bq ; k = x @ wk.T + bk ; v = x @ wv.T + bv   (head slices)
  scores = q k^T / 8 + syn_bias[codons_i, codons_j]
  out    = softmax(scores) @ v ;  final = concat_heads(out) @ wo.T + bo

Key algebraic trick: the pairwise codon bias factors through one-hots,
  pair_bias = onehot @ syn_bias @ onehot.T
so augmenting q' = [(q+bq)/8 | onehot @ syn_bias] and k' = [k | onehot] gives
  scores = q' @ k'.T        (effective head dim 128 — exactly one partition)
Softmax runs without max-subtraction (|scores| <= ~4.3, exp safe in fp32) and
the row-sum l is obtained with a ones-column in v: [O | l] = P @ [v | 1].

Sharding: 8 cores = (batch b in {0,1}) x (head h in {0..3}). Each core runs
the full attention for its (b, h), producing the UNNORMALIZED partial
projection outT = (wo_h @ O_h.T) (256, 4096) plus denominators lT (1, 4096);
the host divides, sums the 4 head partials per batch, transposes, adds bo.

Profile-driven design (trace facts from this hardware):
- PE streams ~0.42 ns/row for 512-row fp16 matmuls; ACT exp is a fixed
  1 col/cycle @1.2GHz (dtype-independent), ~1.0us per [128,1024] tile.
  Phase B is ACT/PE co-bound near 1us per 2-key-tile group.
- All big operands fp16 (same PE rate as fp32r here, half the DMA/SBUF).
- Per-512-column chunk tiles for x/q'/k'/v so the attention stream's
  dependencies are per-chunk: phase B starts as soon as chunk 0 is
  projected instead of waiting for the full QKV phase.
- q and k projections fused into one stationary ([wq|wk] -> 128-wide
  output), halving projection matmuls.
- v is flipped to key-major by the DMA engines' XBAR transpose
  (dma_start_transpose), taking the transposes off the PE entirely.
  v chunks are padded to 96 rows (DV=96) so the [96,128]->[128,96] XBAR
  tiles stay aligned; rows 65..95 are zeros, the PV matmul just carries
  them into unread PSUM partitions.
- Attention stream software-pipelined: scores of group g+1 emitted before
  PV of group g; per-block output projection inside the stream.
"""

import numpy as np

import concourse.mybir as mybir
import concourse.tile as tile
from concourse import bacc
from concourse.bass_utils import run_bass_kernel_spmd


def _ensure_axon_ntff_hook():
    """This image's antenv package lacks axon_hooks, so
    run_bass_kernel_spmd(trace=True) (or BASS_TRACE=1) would die on the
    import. Register a compatible module backed by the libaxon_pjrt C ABI
    so tracing works if a caller requests it."""
    import sys
    try:
        import antenv.axon_hooks  # noqa: F401
        return
    except ImportError:
        pass
    import contextlib
    import ctypes
    import types
    try:
        lib = ctypes.CDLL("/opt/axon/libaxon_pjrt.so")
        has = hasattr(lib, "axon_start_nrt_profile")
    except OSError:
        has = False
    if has:
        lib.axon_start_nrt_profile.argtypes = [ctypes.POINTER(ctypes.c_int64),
                                               ctypes.c_size_t]
        lib.axon_start_nrt_profile.restype = ctypes.c_int64
        lib.axon_stop_nrt_profile.argtypes = [ctypes.c_char_p]
        lib.axon_stop_nrt_profile.restype = ctypes.c_int64

        @contextlib.contextmanager
        def _hook(output_dir, device_ids):
            import jax
            jax.devices()
            if device_ids:
                ids = (ctypes.c_int64 * len(device_ids))(*device_ids)
                rc = lib.axon_start_nrt_profile(ids, len(device_ids))
            else:
                rc = lib.axon_start_nrt_profile(None, 0)
            if rc != 0:
                raise RuntimeError(f"axon_start_nrt_profile rc={rc}")
            try:
                yield
            finally:
                lib.axon_stop_nrt_profile(str(output_dir).encode())
    else:
        _hook = None

    mod = types.ModuleType("antenv.axon_hooks")
    _state = {"hook": _hook}
    mod.get_axon_ntff_profile_hook = lambda: _state["hook"]
    mod.set_axon_ntff_profile_hook = lambda h: _state.__setitem__("hook", h)
    sys.modules["antenv.axon_hooks"] = mod


_ensure_axon_ntff_hook()

B, S, HID, NH, D = 2, 4096, 256, 4, 64
DV = 96            # v rows: 64 v + 1 ones + 31 zero pad (XBAR 16-row tiles)
LCOL = D           # index of the ones column inside a v tile
QB = 512           # query block (free dim of score matmuls)
KT = 128           # key tile (partition dim of transposed scores)
CH = 512           # x chunk width
NCH = S // CH      # 8
NQB = S // QB      # 8
NKT = S // KT      # 32
KPC = CH // KT     # key tiles per chunk = 4
GRP = 2            # key tiles per exp group (2 PSUM banks per group)

F32 = mybir.dt.float32
F16 = mybir.dt.float16
Exp = mybir.ActivationFunctionType.Exp


def build_program():
    nc = bacc.Bacc("TRN2", target_bir_lowering=False, debug=False, num_devices=8)

    def di(name, shape, dt=F16):
        return nc.dram_tensor(name, shape, dt, kind="ExternalInput").ap()

    xT = di("xT", [HID, S])            # x[b].T
    wqkT = di("wqkT", [HID, 2 * D])    # [wq_h.T/8 | wk_h.T]
    wvT = di("wvT", [HID, DV])         # wv_h.T, cols 64..95 zero
    bqk = di("bqk", [2 * D, 1], F32)   # [bq_h/8 ; bk_h]
    bv1 = di("bv1", [DV, 1], F32)      # [bv_h | 1 | 0..] column
    bsynT = di("bsynT", [D, S])        # (onehot @ syn_bias).T
    onehotT = di("onehotT", [D, S])
    woT = di("woT", [D, HID])          # wo[:, hslice].T
    outT = nc.dram_tensor("outT", [HID, S], F32, kind="ExternalOutput").ap()
    lT = nc.dram_tensor("lT", [1, S], F32, kind="ExternalOutput").ap()

    with tile.TileContext(nc) as tc:
        _body(tc, xT, wqkT, wvT, bqk, bv1, bsynT, onehotT, woT, outT, lT)
    nc.compile()
    return nc


def _body(tc, xT, wqkT, wvT, bqk, bv1, bsynT, onehotT, woT, outT, lT):
    nc = tc.nc
    mm = nc.tensor.matmul

    with (
        tc.tile_pool(name="const", bufs=1) as constp,
        tc.tile_pool(name="big", bufs=1) as bigp,
        tc.tile_pool(name="vts", bufs=3) as vtsp,
        tc.tile_pool(name="pt", bufs=6) as ptp,
        tc.tile_pool(name="ob", bufs=2) as obp,
    ):
        # ---- constants ----
        wqk0 = constp.tile([128, 2 * D], F16, name="wqk0", tag="wqk0")
        wqk1 = constp.tile([128, 2 * D], F16, name="wqk1", tag="wqk1")
        wv0 = constp.tile([128, DV], F16, name="wv0", tag="wv0")
        wv1 = constp.tile([128, DV], F16, name="wv1", tag="wv1")
        bqk_sb = constp.tile([2 * D, 1], F32, name="bqk_sb", tag="bqk_sb")
        bv1_sb = constp.tile([DV, 1], F32, name="bv1_sb", tag="bv1_sb")
        wo_sb = constp.tile([D, HID], F16, name="wo_sb", tag="wo_sb")

        # per-chunk activations (dependency granularity = one 512-col chunk)
        xc0 = [bigp.tile([128, CH], F16, name=f"xc0_{c}", tag=f"xc0_{c}")
               for c in range(NCH)]
        xc1 = [bigp.tile([128, CH], F16, name=f"xc1_{c}", tag=f"xc1_{c}")
               for c in range(NCH)]
        qT = [bigp.tile([128, CH], F16, name=f"qT_{c}", tag=f"qT_{c}")
              for c in range(NCH)]   # 0:64 q/8, 64:128 bsynT chunk
        kT = [bigp.tile([128, CH], F16, name=f"kT_{c}", tag=f"kT_{c}")
              for c in range(NCH)]   # 0:64 k,   64:128 onehotT chunk
        vbc = [bigp.tile([128, KPC * DV], F16, name=f"vbc_{c}", tag=f"vbc_{c}")
               for c in range(NCH)]  # v' key-major, 4 tiles of [128, DV]
        oall = bigp.tile([D, S], F16, name="oall", tag="oall")
        l_sb = bigp.tile([1, S], F32, name="l_sb", tag="l_sb")

        # ---- DMA: weights, then per-chunk x + bias streams ----
        nc.sync.dma_start(wqk0[:], wqkT[0:128, :])
        nc.sync.dma_start(wqk1[:], wqkT[128:256, :])
        nc.sync.dma_start(wv0[:], wvT[0:128, :])
        nc.sync.dma_start(wv1[:], wvT[128:256, :])
        nc.sync.dma_start(bqk_sb[:], bqk[:])
        nc.sync.dma_start(bv1_sb[:], bv1[:])
        nc.sync.dma_start(wo_sb[:], woT[:])
        for c in range(NCH):
            cs = slice(c * CH, (c + 1) * CH)
            nc.sync.dma_start(xc0[c][:], xT[0:128, cs])
            nc.sync.dma_start(xc1[c][:], xT[128:256, cs])
            nc.sync.dma_start(qT[c][64:128, :], bsynT[:, cs])
            nc.sync.dma_start(kT[c][64:128, :], onehotT[:, cs])

        # ---- phase A: projections per chunk as DMA lands ----
        with tc.tile_pool(name="psA", bufs=2, space="PSUM") as psA:
            for c in range(NCH):
                qkp = psA.tile([128, CH], F32, name="qkp", tag="qkp")
                mm(qkp[:], wqk0[:], xc0[c][:], start=True, stop=False)
                mm(qkp[:], wqk1[:], xc1[c][:], start=False, stop=True)
                nc.vector.tensor_scalar_add(qT[c][0:D, :], qkp[0:D, :],
                                            bqk_sb[0:D, :])
                nc.vector.tensor_scalar_add(kT[c][0:D, :], qkp[D:2 * D, :],
                                            bqk_sb[D:2 * D, :])

                vtp = psA.tile([DV, CH], F32, name="vtp", tag="vtp")
                mm(vtp[:], wv0[:], xc0[c][:], start=True, stop=False)
                mm(vtp[:], wv1[:], xc1[c][:], start=False, stop=True)
                # bias column [bv | 1 | 0..] adds the ones row during eviction
                vts = vtsp.tile([DV, CH], F16, name="vts", tag="vts")
                nc.vector.tensor_scalar_add(vts[:], vtp[:], bv1_sb[:])
                # flip to key-major with the DMA engines' XBAR transpose
                for j in range(KPC):
                    nc.sync.dma_start_transpose(
                        vbc[c][:, j * DV:(j + 1) * DV],
                        vts[:, j * KT:(j + 1) * KT])

        # ---- phase B: flash attention (ACT/PE co-bound stream) ----
        groups = [list(range(g, min(g + GRP, NKT))) for g in range(0, NKT, GRP)]
        with (
            tc.tile_pool(name="psB", bufs=3, space="PSUM") as psB,
            tc.tile_pool(name="psAcc", bufs=2, space="PSUM") as psAcc,
        ):
            oaccs = {}

            def emit_pv(qb, gi, p3):
                qsl = slice(qb * QB, (qb + 1) * QB)
                if gi == 0:
                    oaccs[qb] = psAcc.tile([DV, QB], F32, name="oacc",
                                           tag="oacc")
                oacc = oaccs[qb]
                for m, j in enumerate(groups[gi]):
                    c, jj = divmod(j, KPC)
                    mm(oacc[:], vbc[c][:, jj * DV:(jj + 1) * DV],
                       p3[:, m * QB:(m + 1) * QB],
                       start=(j == 0), stop=(j == NKT - 1))
                if gi == len(groups) - 1:
                    # stash O^T and l (normalization happens on the host),
                    # then project this block and ship it out
                    nc.vector.tensor_copy(oall[:, qsl], oacc[0:D, :])
                    nc.vector.tensor_copy(l_sb[:, qsl],
                                          oacc[LCOL:LCOL + 1, :])
                    pj = psB.tile([128, 2 * QB], F32, name="pj", tag="s3")
                    mm(pj[:, 0:QB], wo_sb[:, 0:128], oall[:, qsl],
                       start=True, stop=True)
                    mm(pj[:, QB:2 * QB], wo_sb[:, 128:256], oall[:, qsl],
                       start=True, stop=True)
                    ob = obp.tile([128, 2 * QB], F32, name="ob", tag="ob")
                    nc.vector.tensor_copy(ob[:], pj[:])
                    nc.sync.dma_start(outT[0:128, qsl], ob[:, 0:QB])
                    nc.sync.dma_start(outT[128:256, qsl], ob[:, QB:2 * QB])

            pending = None
            for qb in range(NQB):
                for gi, js in enumerate(groups):
                    n = len(js)
                    s3 = psB.tile([128, n * QB], F32, name="s3", tag="s3")
                    for m, j in enumerate(js):
                        c, jj = divmod(j, KPC)
                        mm(s3[:, m * QB:(m + 1) * QB],
                           kT[c][:, jj * KT:(jj + 1) * KT], qT[qb][:],
                           start=True, stop=True)
                    p3 = ptp.tile([128, n * QB], F16, name="p3", tag="p3")
                    nc.scalar.activation(p3[:], s3[:], Exp)
                    if pending is not None:
                        emit_pv(*pending)
                    pending = (qb, gi, p3)
            emit_pv(*pending)

            nc.sync.dma_start(lT[:], l_sb[:])


_NC_CACHE = {}


def _get_program():
    if "nc" not in _NC_CACHE:
        _NC_CACHE["nc"] = build_program()
    return _NC_CACHE["nc"]


def make_in_maps(x, codons, syn_bias, wq, bq, wk, bk, wv, bv, wo):
    in_maps = []
    for core in range(8):
        b, h = divmod(core, NH)
        hsl = slice(h * D, (h + 1) * D)
        cod = codons[b]
        onehotT = np.zeros((D, S), np.float16)
        onehotT[cod, np.arange(S)] = 1.0
        wqk = np.concatenate([wq[hsl, :].T / 8.0, wk[hsl, :].T], axis=1)
        wvp = np.concatenate(
            [wv[hsl, :].T, np.zeros((HID, DV - D), np.float32)], axis=1)
        bv1 = np.concatenate(
            [bv[hsl], [np.float32(1.0)], np.zeros(DV - D - 1, np.float32)])
        in_maps.append({
            "xT": x[b].T.astype(np.float16),
            "wqkT": wqk.astype(np.float16),
            "wvT": wvp.astype(np.float16),
            "bqk": np.concatenate([bq[hsl] / 8.0, bk[hsl]]).reshape(
                2 * D, 1).astype(np.float32),
            "bv1": bv1.reshape(DV, 1).astype(np.float32),
            "bsynT": np.ascontiguousarray(syn_bias.T[:, cod]).astype(np.float16),
            "onehotT": onehotT,
            "woT": wo[:, hsl].T.astype(np.float16),
        })
    return in_maps


def kernel_run(inputs, trace=False):
    x = np.asarray(inputs["x"], np.float32)
    codons = np.asarray(inputs["codons"]).astype(np.int64)
    syn_bias = np.asarray(inputs["syn_bias"], np.float32)
    wq = np.asarray(inputs["wq"], np.float32)
    bq = np.asarray(inputs["bq"], np.float32)
    wk = np.asarray(inputs["wk"], np.float32)
    bk = np.asarray(inputs["bk"], np.float32)
    wv = np.asarray(inputs["wv"], np.float32)
    bv = np.asarray(inputs["bv"], np.float32)
    wo = np.asarray(inputs["wo"], np.float32)
    bo = np.asarray(inputs["bo"], np.float32)

    nc = _get_program()
    in_maps = make_in_maps(x, codons, syn_bias, wq, bq, wk, bk, wv, bv, wo)
    res = run_bass_kernel_spmd(nc, in_maps, core_ids=list(range(8)), trace=trace)

    out = np.empty((B, S, HID), np.float32)
    for b in range(B):
        acc = None
        for h in range(NH):
            r = res.results[NH * b + h]
            part = r["outT"] / r["lT"]          # normalize per head
            acc = part if acc is None else acc + part
        out[b] = acc.T + bo
    return out, res

def kernel(**inputs):
    out, _ = kernel_run(inputs, trace=False)
    return out


# revision 4
# speedup vs baseline: 1.1669x; 1.1669x over previous
"""CodonAttention Trainium2 kernel (fp16 stream, issue-lean pipeline).

Math (per batch b, head h):
  q = x @ wq.T + bq ; k = x @ wk.T + bk ; v = x @ wv.T + bv   (head slices)
  scores = q k^T / 8 + syn_bias[codons_i, codons_j]
  out    = softmax(scores) @ v ;  final = concat_heads(out) @ wo.T + bo

Key algebraic trick: the pairwise codon bias factors through one-hots,
  pair_bias = onehot @ syn_bias @ onehot.T
so augmenting q' = [(q+bq)/8 | onehot @ syn_bias] and k' = [k | onehot] gives
  scores = q' @ k'.T        (effective head dim 128 — exactly one partition)
Softmax runs without max-subtraction (|scores| <= ~4.3, exp safe in fp32) and
the row-sum l is obtained with a ones-column in v: [O | l] = P @ [v | 1].

Sharding: 8 cores = (batch b in {0,1}) x (head h in {0..3}). Each core runs
the full attention for its (b, h), producing the UNNORMALIZED partial
projection outT = (wo_h @ O_h.T) (256, 4096) plus denominators lT (1, 4096);
the host divides, sums the 4 head partials per batch, transposes, adds bo.

Profile-driven design (trace facts from this hardware):
- Phase B is ACT-bound: exp runs 1 col/cycle @1.2GHz regardless of dtype,
  ~1.0us per [128,1024] group; the PE streams 512-row fp16 matmuls at
  ~0.42 ns/row so 4 matmuls/group (~0.87us) fit under the exp.
- Every dma_start costs ~0.6-1us of *issue* time on its queue, so DMA
  issues are spread: Sync + Scalar queues carry x/weights (hwdge),
  GpSimd carries the bias streams and all output DMAs (swdge, idle
  engine). Queue order puts chunk 0 first so compute starts ~3us in.
- Engines execute their queue in order, so late-chunk work must not sit
  in front of the attention stream: q/k projections and v transposes
  for chunks 3..7 are injected INTO the qb-0 attention stream right
  before the groups that consume them.
- The per-block output projection is deferred two groups into the next
  query block so its oacc->oall->PE chain never stalls the score
  pipeline (it runs in loose slots, PSUM bank shared with the qk
  projection pool).
- PSUM budget (8 banks): scores double-buffer 2x2 + oacc 2 + v-flip 1 +
  qkproj/outproj shared 1.
"""

import numpy as np

import concourse.mybir as mybir
import concourse.tile as tile
from concourse import bacc
from concourse.bass_utils import run_bass_kernel_spmd


def _ensure_axon_ntff_hook():
    """This image's antenv package lacks axon_hooks, so
    run_bass_kernel_spmd(trace=True) (or BASS_TRACE=1) would die on the
    import. Register a compatible module backed by the libaxon_pjrt C ABI
    so tracing works if a caller requests it."""
    import sys
    try:
        import antenv.axon_hooks  # noqa: F401
        return
    except ImportError:
        pass
    import contextlib
    import ctypes
    import types
    try:
        lib = ctypes.CDLL("/opt/axon/libaxon_pjrt.so")
        has = hasattr(lib, "axon_start_nrt_profile")
    except OSError:
        has = False
    if has:
        lib.axon_start_nrt_profile.argtypes = [ctypes.POINTER(ctypes.c_int64),
                                               ctypes.c_size_t]
        lib.axon_start_nrt_profile.restype = ctypes.c_int64
        lib.axon_stop_nrt_profile.argtypes = [ctypes.c_char_p]
        lib.axon_stop_nrt_profile.restype = ctypes.c_int64

        @contextlib.contextmanager
        def _hook(output_dir, device_ids):
            import jax
            jax.devices()
            if device_ids:
                ids = (ctypes.c_int64 * len(device_ids))(*device_ids)
                rc = lib.axon_start_nrt_profile(ids, len(device_ids))
            else:
                rc = lib.axon_start_nrt_profile(None, 0)
            if rc != 0:
                raise RuntimeError(f"axon_start_nrt_profile rc={rc}")
            try:
                yield
            finally:
                lib.axon_stop_nrt_profile(str(output_dir).encode())
    else:
        _hook = None

    mod = types.ModuleType("antenv.axon_hooks")
    _state = {"hook": _hook}
    mod.get_axon_ntff_profile_hook = lambda: _state["hook"]
    mod.set_axon_ntff_profile_hook = lambda h: _state.__setitem__("hook", h)
    sys.modules["antenv.axon_hooks"] = mod


_ensure_axon_ntff_hook()

B, S, HID, NH, D = 2, 4096, 256, 4, 64
DV = D + 4         # v cols + ones column + 3 pad
LCOL = D           # index of the ones column inside a v tile
QB = 512           # query block (free dim of score matmuls)
KT = 128           # key tile (partition dim of transposed scores)
CH = 512           # x chunk width
NCH = S // CH      # 8
NQB = S // QB      # 8
NKT = S // KT      # 32
GRP = 2            # key tiles per exp group (2 PSUM banks per group)
NG = NKT // GRP    # 16 groups per query block

F32 = mybir.dt.float32
F32R = mybir.dt.float32r
F16 = mybir.dt.float16
Exp = mybir.ActivationFunctionType.Exp


def build_program():
    nc = bacc.Bacc("TRN2", target_bir_lowering=False, debug=False, num_devices=8)

    def di(name, shape, dt=F16):
        return nc.dram_tensor(name, shape, dt, kind="ExternalInput").ap()

    xT = di("xT", [HID, S])            # x[b].T
    wqk = di("wqk", [128, 256])        # [wqT_hi/8|wkT_hi ; wqT_lo/8|wkT_lo]
    wv2 = di("wv2", [128, 2 * DV])     # [wvT_hi | wvT_lo], col 64.. pad 0
    bias2 = di("bias2", [128, 2], F32) # col0 = [bq/8; bk], col1 = [bv;1;0..]
    bsynT = di("bsynT", [D, S])        # (onehot @ syn_bias).T
    onehotT = di("onehotT", [D, S])
    woT = di("woT", [D, HID])          # wo[:, hslice].T
    idm = di("idm", [128, 128], F32R)  # identity for TensorE transpose
    outT = nc.dram_tensor("outT", [HID, S], F32, kind="ExternalOutput").ap()
    lT = nc.dram_tensor("lT", [1, S], F32, kind="ExternalOutput").ap()

    with tile.TileContext(nc) as tc:
        _body(tc, xT, wqk, wv2, bias2, bsynT, onehotT, woT, idm, outT, lT)
    nc.compile()
    return nc


def _body(tc, xT, wqk, wv2, bias2, bsynT, onehotT, woT, idm, outT, lT):
    nc = tc.nc
    mm = nc.tensor.matmul

    with (
        tc.tile_pool(name="const", bufs=1) as constp,
        tc.tile_pool(name="big", bufs=1) as bigp,
        tc.tile_pool(name="vts", bufs=2) as vtsp,
        tc.tile_pool(name="pt", bufs=6) as ptp,
        tc.tile_pool(name="ob", bufs=2) as obp,
    ):
        # ---- constants ----
        wqk_sb = constp.tile([128, 256], F16, name="wqk_sb", tag="wqk_sb")
        wv_sb = constp.tile([128, 2 * DV], F16, name="wv_sb", tag="wv_sb")
        b2_sb = constp.tile([128, 2], F32, name="b2_sb", tag="b2_sb")
        wo_sb = constp.tile([D, HID], F16, name="wo_sb", tag="wo_sb")
        id_sb = constp.tile([128, 128], F32R, name="id_sb", tag="id_sb")

        # persistent activations (subregion deps make slices per-chunk)
        xT0 = bigp.tile([128, S], F16, name="xT0", tag="xT0")
        xT1 = bigp.tile([128, S], F16, name="xT1", tag="xT1")
        qTt = bigp.tile([128, S], F16, name="qTt", tag="qTt")  # 0:64 q/8, 64:128 bsynT
        kTt = bigp.tile([128, S], F16, name="kTt", tag="kTt")  # 0:64 k,   64:128 onehotT
        vb = bigp.tile([128, NKT * DV], F16, name="vb", tag="vb")  # v' key-major
        oall = bigp.tile([D, S], F16, name="oall", tag="oall")
        l_sb = bigp.tile([1, S], F32, name="l_sb", tag="l_sb")

        # ---- DMA issues, spread across queues, chunk 0 first ----
        # Sync (hwdge): weights+bias+idm then even x chunks
        nc.sync.dma_start(wqk_sb[:], wqk[:])
        nc.sync.dma_start(b2_sb[:], bias2[:])
        nc.sync.dma_start(idm_dst := id_sb[:], idm[:])
        for c in (0, 2, 4, 6):
            cs = slice(c * CH, (c + 1) * CH)
            nc.sync.dma_start(xT0[:, cs], xT[0:128, cs])
            nc.sync.dma_start(xT1[:, cs], xT[128:256, cs])
        # Scalar (hwdge, ACT idle until first exp): wv/wo then odd x chunks
        nc.scalar.dma_start(wv_sb[:], wv2[:])
        nc.scalar.dma_start(wo_sb[:], woT[:])
        for c in (1, 3, 5, 7):
            cs = slice(c * CH, (c + 1) * CH)
            nc.scalar.dma_start(xT0[:, cs], xT[0:128, cs])
            nc.scalar.dma_start(xT1[:, cs], xT[128:256, cs])
        # GpSimd (swdge, idle engine): bias streams per chunk
        for c in range(NCH):
            cs = slice(c * CH, (c + 1) * CH)
            nc.gpsimd.dma_start(qTt[64:128, cs], bsynT[:, cs])
            nc.gpsimd.dma_start(kTt[64:128, cs], onehotT[:, cs])

        with (
            tc.tile_pool(name="psB", bufs=2, space="PSUM") as psB,
            tc.tile_pool(name="psAcc", bufs=2, space="PSUM") as psAcc,
            tc.tile_pool(name="psV", bufs=1, space="PSUM") as psV,
            tc.tile_pool(name="psX", bufs=1, space="PSUM") as psX,  # qkproj+outproj
        ):
            # ---- helpers ----
            def emit_qk(c):
                cs = slice(c * CH, (c + 1) * CH)
                qkp = psX.tile([128, CH], F32, name="qkp", tag="px")
                mm(qkp[:], wqk_sb[:, 0:128], xT0[:, cs], start=True, stop=False)
                mm(qkp[:], wqk_sb[:, 128:256], xT1[:, cs], start=False, stop=True)
                nc.vector.tensor_scalar_add(qTt[0:D, cs], qkp[0:D, :],
                                            b2_sb[0:D, 0:1])
                nc.vector.tensor_scalar_add(kTt[0:D, cs], qkp[D:128, :],
                                            b2_sb[D:128, 0:1])

            def emit_v(c):
                cs = slice(c * CH, (c + 1) * CH)
                vtp = psV.tile([DV, CH], F32, name="vtp", tag="pv")
                mm(vtp[:], wv_sb[:, 0:DV], xT0[:, cs], start=True, stop=False)
                mm(vtp[:], wv_sb[:, DV:2 * DV], xT1[:, cs], start=False,
                   stop=True)
                # bias column [bv | 1 | 0..] adds the ones row during eviction
                vts = vtsp.tile([DV, CH], F32R, name="vts", tag="vts")
                nc.vector.tensor_scalar_add(vts[:], vtp[:], b2_sb[0:DV, 1:2])
                vtr = psV.tile([KT, 4 * DV], F32R, name="vtr", tag="pv")
                for m in range(4):
                    j = 4 * c + m
                    nc.tensor.transpose(vtr[:, m * DV:(m + 1) * DV],
                                        vts[:, m * KT:(m + 1) * KT],
                                        id_sb[0:DV, 0:DV])
                nc.vector.tensor_copy(vb[:, 4 * c * DV:(4 * c + 4) * DV],
                                      vtr[:])

            def proj_steps(qb):
                """Deferred output projection for query block qb, split into
                small steps injected into loose slots of the next block."""
                qsl = slice(qb * QB, (qb + 1) * QB)
                oacc = oaccs.pop(qb)

                def s0():
                    nc.vector.tensor_copy(oall[:, qsl], oacc[0:D, :])
                    nc.vector.tensor_copy(l_sb[:, qsl], oacc[LCOL:LCOL + 1, :])

                def s1():
                    pj = psX.tile([128, QB], F32, name="pj", tag="px")
                    ob = obp.tile([128, QB], F32, name="ob", tag="ob")
                    mm(pj[:], wo_sb[:, 0:128], oall[:, qsl],
                       start=True, stop=True)
                    nc.vector.tensor_copy(ob[:], pj[:])
                    nc.gpsimd.dma_start(outT[0:128, qsl], ob[:])

                def s2():
                    pj = psX.tile([128, QB], F32, name="pj", tag="px")
                    ob = obp.tile([128, QB], F32, name="ob", tag="ob")
                    mm(pj[:], wo_sb[:, 128:256], oall[:, qsl],
                       start=True, stop=True)
                    nc.vector.tensor_copy(ob[:], pj[:])
                    nc.gpsimd.dma_start(outT[128:256, qsl], ob[:])

                return [s0, s1, s2]

            # ---- pre-stream work: chunks 0-2 of qk, 0-1 of v ----
            emit_qk(0)
            emit_v(0)
            emit_qk(1)
            emit_v(1)
            emit_qk(2)

            # injections into the qb-0 stream: (gi -> [thunks])
            inject = {
                0: [lambda: emit_qk(3), lambda: emit_v(2)],
                2: [lambda: emit_qk(4), lambda: emit_v(3)],
                4: [lambda: emit_qk(5), lambda: emit_v(4)],
                6: [lambda: emit_qk(6), lambda: emit_v(5)],
                8: [lambda: emit_qk(7), lambda: emit_v(6)],
                10: [lambda: emit_v(7)],
            }

            # ---- attention stream ----
            oaccs = {}
            pending_pv = None
            pending_proj = []

            def emit_pv(qb, gi, p3):
                if gi == 0:
                    oaccs[qb] = psAcc.tile([DV, QB], F32, name="oacc",
                                           tag="oacc")
                oacc = oaccs[qb]
                for m in range(GRP):
                    j = GRP * gi + m
                    mm(oacc[:], vb[:, j * DV:(j + 1) * DV],
                       p3[:, m * QB:(m + 1) * QB],
                       start=(j == 0), stop=(j == NKT - 1))

            for qb in range(NQB):
                qsl = slice(qb * QB, (qb + 1) * QB)
                for gi in range(NG):
                    if qb == 0:
                        for thunk in inject.get(gi, ()):
                            thunk()
                    s3 = psB.tile([128, GRP * QB], F32, name="s3", tag="s3")
                    for m in range(GRP):
                        j = GRP * gi + m
                        jl = slice(j * KT, (j + 1) * KT)
                        mm(s3[:, m * QB:(m + 1) * QB], kTt[:, jl], qTt[:, qsl],
                           start=True, stop=True)
                    p3 = ptp.tile([128, GRP * QB], F16, name="p3", tag="p3")
                    nc.scalar.activation(p3[:], s3[:], Exp)
                    if pending_pv is not None:
                        emit_pv(*pending_pv)
                    pending_pv = (qb, gi, p3)
                    # one deferred projection step per group, starting 2
                    # groups into the block
                    if pending_proj and gi >= 2:
                        pending_proj.pop(0)()
                if qb > 0:
                    for step in pending_proj:  # leftovers (defensive)
                        step()
                    pending_proj = []
                emit_pv(*pending_pv)
                pending_pv = None
                pending_proj = proj_steps(qb)

            for step in pending_proj:
                step()
            nc.gpsimd.dma_start(lT[:], l_sb[:])


_NC_CACHE = {}


def _get_program():
    if "nc" not in _NC_CACHE:
        _NC_CACHE["nc"] = build_program()
    return _NC_CACHE["nc"]


def make_in_maps(x, codons, syn_bias, wq, bq, wk, bk, wv, bv, wo):
    in_maps = []
    for core in range(8):
        b, h = divmod(core, NH)
        hsl = slice(h * D, (h + 1) * D)
        cod = codons[b]
        onehotT = np.zeros((D, S), np.float16)
        onehotT[cod, np.arange(S)] = 1.0
        # [wqT/8 | wkT] packed as [hi-half ; lo-half] -> [128, 256]
        wqk_full = np.concatenate([wq[hsl, :].T / 8.0, wk[hsl, :].T], axis=1)
        wqk = np.concatenate([wqk_full[0:128], wqk_full[128:256]], axis=1)
        wvp = np.concatenate(
            [wv[hsl, :].T, np.zeros((HID, DV - D), np.float32)], axis=1)
        wv2 = np.concatenate([wvp[0:128], wvp[128:256]], axis=1)
        bias2 = np.zeros((128, 2), np.float32)
        bias2[:, 0] = np.concatenate([bq[hsl] / 8.0, bk[hsl]])
        bias2[0:D, 1] = bv[hsl]
        bias2[LCOL, 1] = 1.0
        in_maps.append({
            "xT": x[b].T.astype(np.float16),
            "wqk": wqk.astype(np.float16),
            "wv2": wv2.astype(np.float16),
            "bias2": bias2,
            "bsynT": np.ascontiguousarray(syn_bias.T[:, cod]).astype(np.float16),
            "onehotT": onehotT,
            "woT": wo[:, hsl].T.astype(np.float16),
            "idm": np.eye(128, dtype=np.float32),
        })
    return in_maps


def kernel_run(inputs, trace=False):
    x = np.asarray(inputs["x"], np.float32)
    codons = np.asarray(inputs["codons"]).astype(np.int64)
    syn_bias = np.asarray(inputs["syn_bias"], np.float32)
    wq = np.asarray(inputs["wq"], np.float32)
    bq = np.asarray(inputs["bq"], np.float32)
    wk = np.asarray(inputs["wk"], np.float32)
    bk = np.asarray(inputs["bk"], np.float32)
    wv = np.asarray(inputs["wv"], np.float32)
    bv = np.asarray(inputs["bv"], np.float32)
    wo = np.asarray(inputs["wo"], np.float32)
    bo = np.asarray(inputs["bo"], np.float32)

    nc = _get_program()
    in_maps = make_in_maps(x, codons, syn_bias, wq, bq, wk, bk, wv, bv, wo)
    res = run_bass_kernel_spmd(nc, in_maps, core_ids=list(range(8)), trace=trace)

    out = np.empty((B, S, HID), np.float32)
    for b in range(B):
        acc = None
        for h in range(NH):
            r = res.results[NH * b + h]
            part = r["outT"] / r["lT"]          # normalize per head
            acc = part if acc is None else acc + part
        out[b] = acc.T + bo
    return out, res


def kernel(**inputs):
    out, _ = kernel_run(inputs, trace=False)
    return out


# revision 6
# speedup vs baseline: 1.1998x; 1.0282x over previous
"""CodonAttention Trainium2 kernel (fp16 stream, issue-lean pipeline).

Math (per batch b, head h):
  q = x @ wq.T + bq ; k = x @ wk.T + bk ; v = x @ wv.T + bv   (head slices)
  scores = q k^T / 8 + syn_bias[codons_i, codons_j]
  out    = softmax(scores) @ v ;  final = concat_heads(out) @ wo.T + bo

Key algebraic trick: the pairwise codon bias factors through one-hots,
  pair_bias = onehot @ syn_bias @ onehot.T
so augmenting q' = [(q+bq)/8 | onehot @ syn_bias] and k' = [k | onehot] gives
  scores = q' @ k'.T        (effective head dim 128 — exactly one partition)
Softmax runs without max-subtraction (|scores| <= ~4.3, exp safe in fp32) and
the row-sum l is obtained with a ones-column in v: [O | l] = P @ [v | 1].

Sharding: 8 cores = (batch b in {0,1}) x (head h in {0..3}). Each core runs
the full attention for its (b, h), producing the UNNORMALIZED partial
projection outT = (wo_h @ O_h.T) (256, 4096) plus denominators lT (1, 4096);
the host divides, sums the 4 head partials per batch, transposes, adds bo.

Profile-driven design (trace facts from this hardware):
- Phase B is ACT-bound: exp runs 1 col/cycle @1.2GHz regardless of dtype,
  ~1.0us per [128,1024] group; the PE streams 512-row fp16 matmuls at
  ~0.42 ns/row so 4 matmuls/group (~0.87us) fit under the exp.
- Every dma_start costs ~0.6-1us of *issue* time on its queue, so DMA
  issues are spread: Sync + Scalar queues carry x/weights (hwdge),
  GpSimd carries the bias streams and all output DMAs (swdge, idle
  engine). Queue order puts chunk 0 first so compute starts ~3us in.
- Engines execute their queue in order, so late-chunk work must not sit
  in front of the attention stream: q/k projections and v transposes
  for chunks 3..7 are injected INTO the qb-0 attention stream right
  before the groups that consume them.
- The per-block output projection is deferred two groups into the next
  query block so its oacc->oall->PE chain never stalls the score
  pipeline (it runs in loose slots, PSUM bank shared with the qk
  projection pool).
- PSUM budget (8 banks): scores double-buffer 2x2 + oacc 2 + v-flip 1 +
  qkproj/outproj shared 1.
"""

import numpy as np

import concourse.mybir as mybir
import concourse.tile as tile
from concourse import bacc
from concourse.bass_utils import run_bass_kernel_spmd


def _ensure_axon_ntff_hook():
    """This image's antenv package lacks axon_hooks, so
    run_bass_kernel_spmd(trace=True) (or BASS_TRACE=1) would die on the
    import. Register a compatible module backed by the libaxon_pjrt C ABI
    so tracing works if a caller requests it."""
    import sys
    try:
        import antenv.axon_hooks  # noqa: F401
        return
    except ImportError:
        pass
    import contextlib
    import ctypes
    import types
    try:
        lib = ctypes.CDLL("/opt/axon/libaxon_pjrt.so")
        has = hasattr(lib, "axon_start_nrt_profile")
    except OSError:
        has = False
    if has:
        lib.axon_start_nrt_profile.argtypes = [ctypes.POINTER(ctypes.c_int64),
                                               ctypes.c_size_t]
        lib.axon_start_nrt_profile.restype = ctypes.c_int64
        lib.axon_stop_nrt_profile.argtypes = [ctypes.c_char_p]
        lib.axon_stop_nrt_profile.restype = ctypes.c_int64

        @contextlib.contextmanager
        def _hook(output_dir, device_ids):
            import jax
            jax.devices()
            if device_ids:
                ids = (ctypes.c_int64 * len(device_ids))(*device_ids)
                rc = lib.axon_start_nrt_profile(ids, len(device_ids))
            else:
                rc = lib.axon_start_nrt_profile(None, 0)
            if rc != 0:
                raise RuntimeError(f"axon_start_nrt_profile rc={rc}")
            try:
                yield
            finally:
                lib.axon_stop_nrt_profile(str(output_dir).encode())
    else:
        _hook = None

    mod = types.ModuleType("antenv.axon_hooks")
    _state = {"hook": _hook}
    mod.get_axon_ntff_profile_hook = lambda: _state["hook"]
    mod.set_axon_ntff_profile_hook = lambda h: _state.__setitem__("hook", h)
    sys.modules["antenv.axon_hooks"] = mod


_ensure_axon_ntff_hook()

B, S, HID, NH, D = 2, 4096, 256, 4, 64
DV = D + 4         # v cols + ones column + 3 pad
LCOL = D           # index of the ones column inside a v tile
QB = 512           # query block (free dim of score matmuls)
KT = 128           # key tile (partition dim of transposed scores)
CH = 512           # x chunk width
NCH = S // CH      # 8
NQB = S // QB      # 8
NKT = S // KT      # 32
GRP = 2            # key tiles per exp group (2 PSUM banks per group)
NG = NKT // GRP    # 16 groups per query block

F32 = mybir.dt.float32
F32R = mybir.dt.float32r
F16 = mybir.dt.float16
Exp = mybir.ActivationFunctionType.Exp


def build_program():
    nc = bacc.Bacc("TRN2", target_bir_lowering=False, debug=False, num_devices=8)

    def di(name, shape, dt=F16):
        return nc.dram_tensor(name, shape, dt, kind="ExternalInput").ap()

    xT = di("xT", [HID, S])            # x[b].T
    wqk = di("wqk", [128, 256])        # [wqT_hi/8|wkT_hi ; wqT_lo/8|wkT_lo]
    wv2 = di("wv2", [128, 2 * DV])     # [wvT_hi | wvT_lo], col 64.. pad 0
    bias2 = di("bias2", [128, 2], F32) # col0 = [bq/8; bk], col1 = [bv;1;0..]
    bsynT = di("bsynT", [D, S])        # (onehot @ syn_bias).T
    onehotT = di("onehotT", [D, S])
    woT = di("woT", [D, HID])          # wo[:, hslice].T
    idm = di("idm", [128, 128], F32R)  # identity for TensorE transpose
    outT = nc.dram_tensor("outT", [HID, S], F16, kind="ExternalOutput").ap()
    lT = nc.dram_tensor("lT", [1, S], F32, kind="ExternalOutput").ap()

    with tile.TileContext(nc) as tc:
        _body(tc, xT, wqk, wv2, bias2, bsynT, onehotT, woT, idm, outT, lT)
    nc.compile()
    return nc


def _body(tc, xT, wqk, wv2, bias2, bsynT, onehotT, woT, idm, outT, lT):
    nc = tc.nc
    mm = nc.tensor.matmul

    with (
        tc.tile_pool(name="const", bufs=1) as constp,
        tc.tile_pool(name="big", bufs=1) as bigp,
        tc.tile_pool(name="vts", bufs=2) as vtsp,
        tc.tile_pool(name="pt", bufs=6) as ptp,
        tc.tile_pool(name="ob", bufs=2) as obp,
    ):
        # ---- constants ----
        wqk_sb = constp.tile([128, 256], F16, name="wqk_sb", tag="wqk_sb")
        wv_sb = constp.tile([128, 2 * DV], F16, name="wv_sb", tag="wv_sb")
        b2_sb = constp.tile([128, 2], F32, name="b2_sb", tag="b2_sb")
        wo_sb = constp.tile([D, HID], F16, name="wo_sb", tag="wo_sb")
        id_sb = constp.tile([128, 128], F32R, name="id_sb", tag="id_sb")

        # persistent activations (subregion deps make slices per-chunk)
        xT0 = bigp.tile([128, S], F16, name="xT0", tag="xT0")
        xT1 = bigp.tile([128, S], F16, name="xT1", tag="xT1")
        qTt = bigp.tile([128, S], F16, name="qTt", tag="qTt")  # 0:64 q/8, 64:128 bsynT
        kTt = bigp.tile([128, S], F16, name="kTt", tag="kTt")  # 0:64 k,   64:128 onehotT
        vb = bigp.tile([128, NKT * DV], F16, name="vb", tag="vb")  # v' key-major
        oall = bigp.tile([D, S], F16, name="oall", tag="oall")
        l_sb = bigp.tile([1, S], F32, name="l_sb", tag="l_sb")

        # ---- DMA issues, spread across queues, chunk 0 first ----
        # Sync (hwdge): weights+bias+idm then even x chunks
        nc.sync.dma_start(wqk_sb[:], wqk[:])
        nc.sync.dma_start(b2_sb[:], bias2[:])
        nc.sync.dma_start(idm_dst := id_sb[:], idm[:])
        for c in (0, 2, 4, 6):
            cs = slice(c * CH, (c + 1) * CH)
            nc.sync.dma_start(xT0[:, cs], xT[0:128, cs])
            nc.sync.dma_start(xT1[:, cs], xT[128:256, cs])
        # Scalar (hwdge, ACT idle until first exp): wv/wo then odd x chunks
        nc.scalar.dma_start(wv_sb[:], wv2[:])
        nc.scalar.dma_start(wo_sb[:], woT[:])
        for c in (1, 3, 5, 7):
            cs = slice(c * CH, (c + 1) * CH)
            nc.scalar.dma_start(xT0[:, cs], xT[0:128, cs])
            nc.scalar.dma_start(xT1[:, cs], xT[128:256, cs])
        # GpSimd (swdge, idle engine): bias streams per chunk
        for c in range(NCH):
            cs = slice(c * CH, (c + 1) * CH)
            nc.gpsimd.dma_start(qTt[64:128, cs], bsynT[:, cs])
            nc.gpsimd.dma_start(kTt[64:128, cs], onehotT[:, cs])

        with (
            tc.tile_pool(name="psB", bufs=3, space="PSUM") as psB,
            tc.tile_pool(name="psAcc", bufs=1, space="PSUM") as psAcc,
            tc.tile_pool(name="psX", bufs=1, space="PSUM") as psX,  # qk/v/outproj
        ):
            psV = psX
            # ---- helpers ----
            def emit_qk(c):
                cs = slice(c * CH, (c + 1) * CH)
                qkp = psX.tile([128, CH], F32, name="qkp", tag="px")
                mm(qkp[:], wqk_sb[:, 0:128], xT0[:, cs], start=True, stop=False)
                mm(qkp[:], wqk_sb[:, 128:256], xT1[:, cs], start=False, stop=True)
                nc.vector.tensor_scalar_add(qTt[0:D, cs], qkp[0:D, :],
                                            b2_sb[0:D, 0:1])
                nc.vector.tensor_scalar_add(kTt[0:D, cs], qkp[D:128, :],
                                            b2_sb[D:128, 0:1])

            def emit_v(c):
                cs = slice(c * CH, (c + 1) * CH)
                vtp = psV.tile([DV, CH], F32, name="vtp", tag="px")
                mm(vtp[:], wv_sb[:, 0:DV], xT0[:, cs], start=True, stop=False)
                mm(vtp[:], wv_sb[:, DV:2 * DV], xT1[:, cs], start=False,
                   stop=True)
                # bias column [bv | 1 | 0..] adds the ones row during eviction
                vts = vtsp.tile([DV, CH], F32R, name="vts", tag="vts")
                nc.vector.tensor_scalar_add(vts[:], vtp[:], b2_sb[0:DV, 1:2])
                vtr = psV.tile([KT, 4 * DV], F32R, name="vtr", tag="px")
                for m in range(4):
                    j = 4 * c + m
                    nc.tensor.transpose(vtr[:, m * DV:(m + 1) * DV],
                                        vts[:, m * KT:(m + 1) * KT],
                                        id_sb[0:DV, 0:DV])
                nc.vector.tensor_copy(vb[:, 4 * c * DV:(4 * c + 4) * DV],
                                      vtr[:])

            def proj_steps(qb):
                """Deferred output projection for query block qb, split into
                small steps injected into loose slots of the next block."""
                qsl = slice(qb * QB, (qb + 1) * QB)
                oacc = oaccs.pop(qb)
                # evict now (DVE is idle) so the single oacc slot frees fast
                nc.vector.tensor_copy(oall[:, qsl], oacc[0:D, :])
                nc.vector.tensor_copy(l_sb[:, qsl], oacc[LCOL:LCOL + 1, :])

                def s1():
                    pj = psX.tile([128, QB], F32, name="pj", tag="px")
                    ob = obp.tile([128, QB], F16, name="ob", tag="ob")
                    mm(pj[:], wo_sb[:, 0:128], oall[:, qsl],
                       start=True, stop=True)
                    nc.vector.tensor_copy(ob[:], pj[:])
                    nc.gpsimd.dma_start(outT[0:128, qsl], ob[:])

                def s2():
                    pj = psX.tile([128, QB], F32, name="pj", tag="px")
                    ob = obp.tile([128, QB], F16, name="ob", tag="ob")
                    mm(pj[:], wo_sb[:, 128:256], oall[:, qsl],
                       start=True, stop=True)
                    nc.vector.tensor_copy(ob[:], pj[:])
                    nc.gpsimd.dma_start(outT[128:256, qsl], ob[:])

                return [s1, s2]

            # ---- pre-stream work: qk chunks 0-2, v chunk 0 ----
            emit_qk(0)
            emit_qk(1)
            emit_v(0)
            emit_qk(2)

            # injections into the qb-0 stream: late enough that the chunk
            # DMA has landed (no PE-queue head-of-line wait), early enough
            # to beat consumption (scores need qk(c) at gi=2c, PV needs
            # v(c) at gi=2c+1)
            inject = {
                1: [lambda: emit_v(1)],
                3: [lambda: emit_v(2)],
                4: [lambda: emit_qk(3)],
                5: [lambda: emit_v(3)],
                6: [lambda: emit_qk(4)],
                7: [lambda: emit_v(4)],
                8: [lambda: emit_qk(5)],
                9: [lambda: emit_v(5)],
                10: [lambda: emit_qk(6)],
                11: [lambda: emit_v(6)],
                12: [lambda: emit_qk(7)],
                13: [lambda: emit_v(7)],
            }

            # ---- attention stream ----
            oaccs = {}
            pending_pv = None
            pending_proj = []

            def emit_pv(qb, gi, p3):
                if gi == 0:
                    oaccs[qb] = psAcc.tile([DV, QB], F32, name="oacc",
                                           tag="oacc")
                oacc = oaccs[qb]
                for m in range(GRP):
                    j = GRP * gi + m
                    mm(oacc[:], vb[:, j * DV:(j + 1) * DV],
                       p3[:, m * QB:(m + 1) * QB],
                       start=(j == 0), stop=(j == NKT - 1))

            for qb in range(NQB):
                qsl = slice(qb * QB, (qb + 1) * QB)
                for gi in range(NG):
                    if qb == 0:
                        for thunk in inject.get(gi, ()):
                            thunk()
                    s3 = psB.tile([128, GRP * QB], F32, name="s3", tag="s3")
                    for m in range(GRP):
                        j = GRP * gi + m
                        jl = slice(j * KT, (j + 1) * KT)
                        mm(s3[:, m * QB:(m + 1) * QB], kTt[:, jl], qTt[:, qsl],
                           start=True, stop=True)
                    p3 = ptp.tile([128, GRP * QB], F16, name="p3", tag="p3")
                    nc.scalar.activation(p3[:], s3[:], Exp)
                    if pending_pv is not None:
                        emit_pv(*pending_pv)
                    pending_pv = (qb, gi, p3)
                    # deferred projection matmuls in loose slots
                    if pending_proj and gi in (4, 10):
                        pending_proj.pop(0)()
                if qb > 0:
                    for step in pending_proj:  # leftovers (defensive)
                        step()
                    pending_proj = []
                emit_pv(*pending_pv)
                pending_pv = None
                pending_proj = proj_steps(qb)

            for step in pending_proj:
                step()
            nc.gpsimd.dma_start(lT[:], l_sb[:])


_NC_CACHE = {}


def _get_program():
    if "nc" not in _NC_CACHE:
        _NC_CACHE["nc"] = build_program()
    return _NC_CACHE["nc"]


def make_in_maps(x, codons, syn_bias, wq, bq, wk, bk, wv, bv, wo):
    in_maps = []
    for core in range(8):
        b, h = divmod(core, NH)
        hsl = slice(h * D, (h + 1) * D)
        cod = codons[b]
        onehotT = np.zeros((D, S), np.float16)
        onehotT[cod, np.arange(S)] = 1.0
        # [wqT/8 | wkT] packed as [hi-half ; lo-half] -> [128, 256]
        wqk_full = np.concatenate([wq[hsl, :].T / 8.0, wk[hsl, :].T], axis=1)
        wqk = np.concatenate([wqk_full[0:128], wqk_full[128:256]], axis=1)
        wvp = np.concatenate(
            [wv[hsl, :].T, np.zeros((HID, DV - D), np.float32)], axis=1)
        wv2 = np.concatenate([wvp[0:128], wvp[128:256]], axis=1)
        bias2 = np.zeros((128, 2), np.float32)
        bias2[:, 0] = np.concatenate([bq[hsl] / 8.0, bk[hsl]])
        bias2[0:D, 1] = bv[hsl]
        bias2[LCOL, 1] = 1.0
        in_maps.append({
            "xT": x[b].T.astype(np.float16),
            "wqk": wqk.astype(np.float16),
            "wv2": wv2.astype(np.float16),
            "bias2": bias2,
            "bsynT": np.ascontiguousarray(syn_bias.T[:, cod]).astype(np.float16),
            "onehotT": onehotT,
            "woT": wo[:, hsl].T.astype(np.float16),
            "idm": np.eye(128, dtype=np.float32),
        })
    return in_maps


def kernel_run(inputs, trace=False):
    x = np.asarray(inputs["x"], np.float32)
    codons = np.asarray(inputs["codons"]).astype(np.int64)
    syn_bias = np.asarray(inputs["syn_bias"], np.float32)
    wq = np.asarray(inputs["wq"], np.float32)
    bq = np.asarray(inputs["bq"], np.float32)
    wk = np.asarray(inputs["wk"], np.float32)
    bk = np.asarray(inputs["bk"], np.float32)
    wv = np.asarray(inputs["wv"], np.float32)
    bv = np.asarray(inputs["bv"], np.float32)
    wo = np.asarray(inputs["wo"], np.float32)
    bo = np.asarray(inputs["bo"], np.float32)

    nc = _get_program()
    in_maps = make_in_maps(x, codons, syn_bias, wq, bq, wk, bk, wv, bv, wo)
    res = run_bass_kernel_spmd(nc, in_maps, core_ids=list(range(8)), trace=trace)

    out = np.empty((B, S, HID), np.float32)
    for b in range(B):
        acc = None
        for h in range(NH):
            r = res.results[NH * b + h]
            part = r["outT"].astype(np.float32) / r["lT"]   # normalize per head
            acc = part if acc is None else acc + part
        out[b] = acc.T + bo
    return out, res


def kernel(**inputs):
    out, _ = kernel_run(inputs, trace=False)
    return out
